# revision 2
# baseline (speedup 1.0000x reference)
"""KMeans VQ-codebook assignment kernel for Trainium2 (8 NeuronCores, SPMD).

Math (matches the jax reference):
    dist[k, n] = ||c_k||^2 + ||x_n||^2 - 2 <c_k, x_n>
    ids[n]     = argmin_k dist[k, n]        (first index on ties)
    x_cent[n]  = centroids[ids[n]]

Device formulation: ids[n] = argmax_k score[n, k] where
    score[n, k] = <x_n, c_k> - ||c_k||^2 / 2
(the ||x_n||^2 term is constant per n and cannot change the argmax).

The dot product runs as three fp16 matmul terms via a hi/lo split
(x = xh + xl, c = ch + cl; dot = xh ch + xh cl + xl ch, dropping the
~2^-22-relative xl cl term): fp32 matmuls cost 4 cycles/row on the PE
while fp16 costs 1, so six fp16 passes (3 terms x 2 d-tiles) beat the
fp32 equivalent (8 effective passes) by 25%, at ~1e-5 absolute score
error — an order below the ~1e-4 minimum top-2 score gap of this data
distribution, so the argmax is unchanged vs an fp32 evaluation.

Sharding: data-parallel over N. Each of the 8 cores gets N/8 points and a
replicated codebook (shipped pre-transposed and hi/lo-split) plus
c2rep = ||c||^2/2 broadcast to [128, 2048].

Per 128-point tile on each core:
  PE    : transpose xh/xl tiles to [d, n] (identity matmul), then 4 k-chunks
          x 6 fp16 matmuls accumulating score chunks in fp32 PSUM
  ACT   : evict PSUM chunks to SBUF
  DVE   : subtract c2rep (first half), reduce_max, max_index
  GPSIMD: subtract c2rep (second half), indirect-DMA gather of the winning
          centroid rows
"""

import sys

sys.path.insert(0, "/opt/trn_rl_repo")

import numpy as np

import concourse.bacc as bacc
import concourse.bass as bass
import concourse.mybir as mybir
import concourse.tile as tile
from concourse.bass_utils import run_bass_kernel_spmd

N_FULL, K, D = 131072, 2048, 256
N_CORES = 8
P = 128          # partitions / points per tile
CHUNK = 512      # k-chunk width (one PSUM bank of fp32)
N_CHUNKS = K // CHUNK
N_DTILES = D // P
BATCH = 8        # n-tiles per gather/output batch

F32 = mybir.dt.float32
F16 = mybir.dt.float16
U32 = mybir.dt.uint32


def build_kernel(n_pts: int):
    """Build the SPMD Bass program for one core processing n_pts points."""
    n_tiles = n_pts // P
    n_batches = max(n_tiles // BATCH, 1)
    batch = min(BATCH, n_tiles)

    nc = bacc.Bacc("TRN2", target_bir_lowering=False, debug=False)

    xh_d = nc.dram_tensor("xh", [n_pts, D], F16, kind="ExternalInput")
    xl_d = nc.dram_tensor("xl", [n_pts, D], F16, kind="ExternalInput")
    cth_d = nc.dram_tensor("cth", [D, K], F16, kind="ExternalInput")   # hi(C)^T
    ctl_d = nc.dram_tensor("ctl", [D, K], F16, kind="ExternalInput")   # lo(C)^T
    c2_d = nc.dram_tensor("c2rep", [P, K], F32, kind="ExternalInput")  # ||c||^2/2 bcast
    id_d = nc.dram_tensor("ident", [P, P], F16, kind="ExternalInput")
    cent_d = nc.dram_tensor("cent", [K, D], F32, kind="ExternalInput")  # gather table
    ids_d = nc.dram_tensor("ids", [n_pts], U32, kind="ExternalOutput")
    xc_d = nc.dram_tensor("xc", [n_pts, D], F32, kind="ExternalOutput")

    xh_r = xh_d.rearrange("(t p) d -> t p d", p=P)
    xl_r = xl_d.rearrange("(t p) d -> t p d", p=P)
    xc_r = xc_d.rearrange("(t p) d -> t p d", p=P)
    ids_r = ids_d.rearrange("(t p) -> p t", p=P)

    with tile.TileContext(nc) as tc:
        with (
            tc.tile_pool(name="const", bufs=1) as cpool,
            tc.tile_pool(name="xload", bufs=3) as xpool,
            tc.tile_pool(name="xtp", bufs=2, space="PSUM") as xtp,
            tc.tile_pool(name="xts", bufs=3) as xts,
            tc.tile_pool(name="sp", bufs=6, space="PSUM") as spsum,
            tc.tile_pool(name="ss", bufs=3) as sspool,
            tc.tile_pool(name="red", bufs=2) as redpool,
            tc.tile_pool(name="idsb", bufs=2) as idspool,
            tc.tile_pool(name="xcg", bufs=3) as xcpool,
        ):
            cth0 = cpool.tile([P, K], F16, tag="cth0")
            cth1 = cpool.tile([P, K], F16, tag="cth1")
            ctl0 = cpool.tile([P, K], F16, tag="ctl0")
            ctl1 = cpool.tile([P, K], F16, tag="ctl1")
            c2rep = cpool.tile([P, K], F32, tag="c2rep")
            ident = cpool.tile([P, P], F16, tag="ident")
            nc.sync.dma_start(cth0[:], cth_d[0:P, :])
            nc.sync.dma_start(cth1[:], cth_d[P : 2 * P, :])
            nc.sync.dma_start(ctl0[:], ctl_d[0:P, :])
            nc.sync.dma_start(ctl1[:], ctl_d[P : 2 * P, :])
            nc.sync.dma_start(c2rep[:], c2_d[:])
            nc.sync.dma_start(ident[:], id_d[:])

            for b in range(n_batches):
                idsb = idspool.tile([P, batch], U32, tag="idsb")
                for j in range(batch):
                    t = b * batch + j
                    # ---- load + transpose xh/xl tiles -> xT [d, n] fp16 ----
                    xin = xpool.tile([P, 2 * D], F16, tag="xin")
                    nc.sync.dma_start(xin[:, 0:D], xh_r[t])
                    nc.sync.dma_start(xin[:, D : 2 * D], xl_r[t])
                    xt_ps = xtp.tile([P, 2 * D], F16, tag="xtp")
                    for blk in range(2 * N_DTILES):
                        sl = slice(blk * P, (blk + 1) * P)
                        nc.tensor.transpose(xt_ps[:, sl], xin[:, sl], ident[:])
                    xT = xts.tile([P, 2 * D], F16, tag="xts")
                    nc.scalar.copy(xT[:], xt_ps[:])
                    # xT cols: [0:128]=xh d0, [128:256]=xh d1,
                    #          [256:384]=xl d0, [384:512]=xl d1

                    # ---- score chunks: 3-term fp16 accumulation ----
                    sc = sspool.tile([P, K], F32, tag="score")
                    for c in range(N_CHUNKS):
                        ps = spsum.tile([P, CHUNK], F32, tag="sps")
                        ksl = slice(c * CHUNK, (c + 1) * CHUNK)
                        nc.tensor.matmul(ps[:], xT[:, 0:P], cth0[:, ksl], start=True, stop=False)
                        nc.tensor.matmul(ps[:], xT[:, P : 2 * P], cth1[:, ksl], start=False, stop=False)
                        nc.tensor.matmul(ps[:], xT[:, 0:P], ctl0[:, ksl], start=False, stop=False)
                        nc.tensor.matmul(ps[:], xT[:, P : 2 * P], ctl1[:, ksl], start=False, stop=False)
                        nc.tensor.matmul(ps[:], xT[:, 2 * P : 3 * P], cth0[:, ksl], start=False, stop=False)
                        nc.tensor.matmul(ps[:], xT[:, 3 * P : 4 * P], cth1[:, ksl], start=False, stop=True)
                        nc.scalar.copy(sc[:, ksl], ps[:])

                    # ---- bias + argmax along k ----
                    half = K // 2
                    nc.vector.tensor_tensor(
                        out=sc[:, 0:half], in0=sc[:, 0:half], in1=c2rep[:, 0:half],
                        op=mybir.AluOpType.subtract,
                    )
                    nc.gpsimd.tensor_tensor(
                        out=sc[:, half:K], in0=sc[:, half:K], in1=c2rep[:, half:K],
                        op=mybir.AluOpType.subtract,
                    )
                    max8 = redpool.tile([P, 8], F32, tag="max8")
                    nc.vector.tensor_reduce(
                        out=max8[:, 0:1], in_=sc[:], axis=mybir.AxisListType.X,
                        op=mybir.AluOpType.max,
                    )
                    nc.vector.tensor_copy(
                        out=max8[:, 1:8], in_=max8[:, 0:1].to_broadcast([P, 7])
                    )
                    idx8 = redpool.tile([P, 8], U32, tag="idx8")
                    nc.vector.max_index(idx8[:], max8[:], sc[:])
                    nc.vector.tensor_copy(out=idsb[:, j : j + 1], in_=idx8[:, 0:1])

                # ---- batch tail: ids out + gather + xc out ----
                nc.sync.dma_start(ids_r[:, b * batch : (b + 1) * batch], idsb[:])
                for j in range(batch):
                    t = b * batch + j
                    xcg = xcpool.tile([P, D], F32, tag="xcg")
                    nc.gpsimd.indirect_dma_start(
                        out=xcg[:],
                        out_offset=None,
                        in_=cent_d[:],
                        in_offset=bass.IndirectOffsetOnAxis(
                            ap=idsb[:, j : j + 1], axis=0
                        ),
                    )
                    nc.sync.dma_start(xc_r[t], xcg[:])

    nc.finalize()
    return nc


_NC_CACHE = {}


def _get_nc(n_pts):
    if n_pts not in _NC_CACHE:
        _NC_CACHE[n_pts] = build_kernel(n_pts)
    return _NC_CACHE[n_pts]


def _hilo(a):
    """fp16 hi/lo split: a ~= hi + lo with ~2^-22 relative error."""
    hi = a.astype(np.float16)
    lo = (a - hi.astype(np.float32)).astype(np.float16)
    return hi, lo


def run(x, centroids, n_cores=N_CORES, trace=False):
    """Shard, run on the NeuronCores, gather. Returns (ids int32, x_cent f32)."""
    x = np.ascontiguousarray(x, dtype=np.float32)
    centroids = np.ascontiguousarray(centroids, dtype=np.float32)
    n = x.shape[0]
    n_pts = n // n_cores
    nc = _get_nc(n_pts)

    xh, xl = _hilo(x)
    ch, cl = _hilo(centroids)
    cth = np.ascontiguousarray(ch.T)
    ctl = np.ascontiguousarray(cl.T)
    c2rep = np.ascontiguousarray(
        np.broadcast_to((centroids.astype(np.float64) ** 2).sum(1).astype(np.float32)
                        * 0.5, (P, K))
    )
    ident = np.eye(P, dtype=np.float16)
    in_maps = [
        {
            "xh": np.ascontiguousarray(xh[i * n_pts : (i + 1) * n_pts]),
            "xl": np.ascontiguousarray(xl[i * n_pts : (i + 1) * n_pts]),
            "cth": cth,
            "ctl": ctl,
            "c2rep": c2rep,
            "ident": ident,
            "cent": centroids,
        }
        for i in range(n_cores)
    ]
    res = run_bass_kernel_spmd(
        nc, in_maps, core_ids=list(range(n_cores)), trace=trace
    )
    ids = np.concatenate([r["ids"] for r in res.results]).astype(np.int32)
    xc = np.concatenate([r["xc"] for r in res.results])
    return (ids, xc), res


def kernel(x, centroids):
    (ids, xc), _ = run(x, centroids)
    return ids, xc


# revision 3
# speedup vs baseline: 1.0674x; 1.0674x over previous
"""KMeans VQ-codebook assignment kernel for Trainium2 (8 NeuronCores, SPMD).

Math (matches the jax reference):
    dist[k, n] = ||c_k||^2 + ||x_n||^2 - 2 <c_k, x_n>
    ids[n]     = argmin_k dist[k, n]        (first index on ties)
    x_cent[n]  = centroids[ids[n]]

Device formulation: ids[n] = argmax_k score[n, k] where
    score[n, k] = <x_n, c_k> - ||c_k||^2 / 2
(the ||x_n||^2 term is constant per n and cannot change the argmax).

The dot product runs as three fp16 matmul terms via a hi/lo split
(x = xh + xl, c = ch + cl; dot = xh ch + xh cl + xl ch, dropping the
~2^-22-relative xl cl term): fp32 matmuls cost 4 cycles/row on the PE
while fp16 costs 1, so six fp16 passes (3 terms x 2 d-tiles) beat the
fp32 equivalent (8 effective passes) by 25%, at ~1e-5 absolute score
error — an order below the ~1e-4 minimum top-2 score gap of this data
distribution, so the argmax is unchanged vs an fp32 evaluation.

Sharding: data-parallel over N. Each of the 8 cores gets N/8 points and a
replicated codebook (shipped pre-transposed and hi/lo-split) plus
c2rep = ||c||^2/2 broadcast to [128, 2048].

Per 128-point tile on each core:
  PE    : transpose xh/xl tiles to [d, n] (identity matmul), then 4 k-chunks
          x 6 fp16 matmuls accumulating score chunks in fp32 PSUM
  ACT   : evict PSUM chunks to SBUF
  DVE   : subtract c2rep (first half), reduce_max, max_index
  GPSIMD: subtract c2rep (second half), indirect-DMA gather of the winning
          centroid rows
"""

import sys

sys.path.insert(0, "/opt/trn_rl_repo")

import numpy as np

import concourse.bacc as bacc
import concourse.bass as bass
import concourse.mybir as mybir
import concourse.tile as tile
from concourse.bass_utils import run_bass_kernel_spmd

N_FULL, K, D = 131072, 2048, 256
N_CORES = 8
P = 128          # partitions / points per tile
CHUNK = 512      # k-chunk width (one PSUM bank of fp32)
N_CHUNKS = K // CHUNK
N_DTILES = D // P
BATCH = 8        # n-tiles per gather/output batch

F32 = mybir.dt.float32
F16 = mybir.dt.float16
U32 = mybir.dt.uint32


def build_kernel(n_pts: int):
    """Build the SPMD Bass program for one core processing n_pts points."""
    n_tiles = n_pts // P
    n_batches = max(n_tiles // BATCH, 1)
    batch = min(BATCH, n_tiles)

    nc = bacc.Bacc("TRN2", target_bir_lowering=False, debug=False)

    xh_d = nc.dram_tensor("xh", [n_pts, D], F16, kind="ExternalInput")
    xl_d = nc.dram_tensor("xl", [n_pts, D], F16, kind="ExternalInput")
    cth_d = nc.dram_tensor("cth", [D, K], F16, kind="ExternalInput")   # hi(C)^T
    ctl_d = nc.dram_tensor("ctl", [D, K], F16, kind="ExternalInput")   # lo(C)^T
    c2_d = nc.dram_tensor("c2rep", [P, K], F32, kind="ExternalInput")  # ||c||^2/2 bcast
    id_d = nc.dram_tensor("ident", [P, P], F16, kind="ExternalInput")
    cent_d = nc.dram_tensor("cent", [K, D], F32, kind="ExternalInput")  # gather table
    ids_d = nc.dram_tensor("ids", [n_pts], U32, kind="ExternalOutput")
    xc_d = nc.dram_tensor("xc", [n_pts, D], F32, kind="ExternalOutput")

    xh_r = xh_d.rearrange("(t p) d -> t p d", p=P)
    xl_r = xl_d.rearrange("(t p) d -> t p d", p=P)
    xc_r = xc_d.rearrange("(t p) d -> t p d", p=P)
    ids_r = ids_d.rearrange("(t p) -> p t", p=P)

    with tile.TileContext(nc) as tc:
        with (
            tc.tile_pool(name="const", bufs=1) as cpool,
            tc.tile_pool(name="xload", bufs=3) as xpool,
            tc.tile_pool(name="xtp", bufs=2, space="PSUM") as xtp,
            tc.tile_pool(name="xts", bufs=3) as xts,
            tc.tile_pool(name="sp", bufs=6, space="PSUM") as spsum,
            tc.tile_pool(name="ss", bufs=3) as sspool,
            tc.tile_pool(name="red", bufs=4) as redpool,
            tc.tile_pool(name="idsb", bufs=2) as idspool,
            tc.tile_pool(name="xcg", bufs=3) as xcpool,
        ):
            cth0 = cpool.tile([P, K], F16, tag="cth0")
            cth1 = cpool.tile([P, K], F16, tag="cth1")
            ctl0 = cpool.tile([P, K], F16, tag="ctl0")
            ctl1 = cpool.tile([P, K], F16, tag="ctl1")
            c2rep = cpool.tile([P, K], F32, tag="c2rep")
            ident = cpool.tile([P, P], F16, tag="ident")
            nc.sync.dma_start(cth0[:], cth_d[0:P, :])
            nc.sync.dma_start(cth1[:], cth_d[P : 2 * P, :])
            nc.sync.dma_start(ctl0[:], ctl_d[0:P, :])
            nc.sync.dma_start(ctl1[:], ctl_d[P : 2 * P, :])
            nc.sync.dma_start(c2rep[:], c2_d[:])
            nc.sync.dma_start(ident[:], id_d[:])

            for b in range(n_batches):
                idsb = idspool.tile([P, batch], U32, tag="idsb")
                for j in range(batch):
                    t = b * batch + j
                    # ---- load + transpose xh/xl tiles -> xT [d, n] fp16 ----
                    xin = xpool.tile([P, 2 * D], F16, tag="xin")
                    nc.sync.dma_start(xin[:, 0:D], xh_r[t])
                    nc.sync.dma_start(xin[:, D : 2 * D], xl_r[t])
                    xt_ps = xtp.tile([P, 2 * D], F16, tag="xtp")
                    for blk in range(2 * N_DTILES):
                        sl = slice(blk * P, (blk + 1) * P)
                        nc.tensor.transpose(xt_ps[:, sl], xin[:, sl], ident[:])
                    xT = xts.tile([P, 2 * D], F16, tag="xts")
                    nc.scalar.copy(xT[:], xt_ps[:])
                    # xT cols: [0:128]=xh d0, [128:256]=xh d1,
                    #          [256:384]=xl d0, [384:512]=xl d1

                    # ---- score chunks: 3-term fp16 accumulation ----
                    sc = sspool.tile([P, K], F32, tag="score")
                    for c in range(N_CHUNKS):
                        ps = spsum.tile([P, CHUNK], F32, tag="sps")
                        ksl = slice(c * CHUNK, (c + 1) * CHUNK)
                        nc.tensor.matmul(ps[:], xT[:, 0:P], cth0[:, ksl], start=True, stop=False)
                        nc.tensor.matmul(ps[:], xT[:, P : 2 * P], cth1[:, ksl], start=False, stop=False)
                        nc.tensor.matmul(ps[:], xT[:, 0:P], ctl0[:, ksl], start=False, stop=False)
                        nc.tensor.matmul(ps[:], xT[:, P : 2 * P], ctl1[:, ksl], start=False, stop=False)
                        nc.tensor.matmul(ps[:], xT[:, 2 * P : 3 * P], cth0[:, ksl], start=False, stop=False)
                        nc.tensor.matmul(ps[:], xT[:, 3 * P : 4 * P], cth1[:, ksl], start=False, stop=True)
                        nc.scalar.copy(sc[:, ksl], ps[:])

                    # ---- bias + argmax along k ----
                    # DVE carries reduce+max_index (its exclusive ops), so the
                    # bias subtract leans on the otherwise-idle gpsimd: 768
                    # columns on DVE, 1280 on gpsimd.
                    cut = 768
                    nc.vector.tensor_tensor(
                        out=sc[:, 0:cut], in0=sc[:, 0:cut], in1=c2rep[:, 0:cut],
                        op=mybir.AluOpType.subtract,
                    )
                    nc.gpsimd.tensor_tensor(
                        out=sc[:, cut:K], in0=sc[:, cut:K], in1=c2rep[:, cut:K],
                        op=mybir.AluOpType.subtract,
                    )
                    max8 = redpool.tile([P, 8], F32, tag="max8")
                    nc.vector.tensor_reduce(
                        out=max8[:, 0:1], in_=sc[:], axis=mybir.AxisListType.X,
                        op=mybir.AluOpType.max,
                    )
                    nc.scalar.copy(
                        out=max8[:, 1:8], in_=max8[:, 0:1].to_broadcast([P, 7])
                    )
                    idx8 = redpool.tile([P, 8], U32, tag="idx8")
                    nc.vector.max_index(idx8[:], max8[:], sc[:])
                    nc.gpsimd.tensor_copy(out=idsb[:, j : j + 1], in_=idx8[:, 0:1])

                # ---- batch tail: ids out + gather + xc out ----
                nc.sync.dma_start(ids_r[:, b * batch : (b + 1) * batch], idsb[:])
                for j in range(batch):
                    t = b * batch + j
                    xcg = xcpool.tile([P, D], F32, tag="xcg")
                    nc.gpsimd.indirect_dma_start(
                        out=xcg[:],
                        out_offset=None,
                        in_=cent_d[:],
                        in_offset=bass.IndirectOffsetOnAxis(
                            ap=idsb[:, j : j + 1], axis=0
                        ),
                    )
                    nc.sync.dma_start(xc_r[t], xcg[:])

    nc.finalize()
    return nc


_NC_CACHE = {}


def _get_nc(n_pts):
    if n_pts not in _NC_CACHE:
        _NC_CACHE[n_pts] = build_kernel(n_pts)
    return _NC_CACHE[n_pts]


def _hilo(a):
    """fp16 hi/lo split: a ~= hi + lo with ~2^-22 relative error."""
    hi = a.astype(np.float16)
    lo = (a - hi.astype(np.float32)).astype(np.float16)
    return hi, lo


def run(x, centroids, n_cores=N_CORES, trace=False):
    """Shard, run on the NeuronCores, gather. Returns (ids int32, x_cent f32)."""
    x = np.ascontiguousarray(x, dtype=np.float32)
    centroids = np.ascontiguousarray(centroids, dtype=np.float32)
    n = x.shape[0]
    n_pts = n // n_cores
    nc = _get_nc(n_pts)

    xh, xl = _hilo(x)
    ch, cl = _hilo(centroids)
    cth = np.ascontiguousarray(ch.T)
    ctl = np.ascontiguousarray(cl.T)
    c2rep = np.ascontiguousarray(
        np.broadcast_to((centroids.astype(np.float64) ** 2).sum(1).astype(np.float32)
                        * 0.5, (P, K))
    )
    ident = np.eye(P, dtype=np.float16)
    in_maps = [
        {
            "xh": np.ascontiguousarray(xh[i * n_pts : (i + 1) * n_pts]),
            "xl": np.ascontiguousarray(xl[i * n_pts : (i + 1) * n_pts]),
            "cth": cth,
            "ctl": ctl,
            "c2rep": c2rep,
            "ident": ident,
            "cent": centroids,
        }
        for i in range(n_cores)
    ]
    res = run_bass_kernel_spmd(
        nc, in_maps, core_ids=list(range(n_cores)), trace=trace
    )
    ids = np.concatenate([r["ids"] for r in res.results]).astype(np.int32)
    xc = np.concatenate([r["xc"] for r in res.results])
    return (ids, xc), res


def kernel(x, centroids):
    (ids, xc), _ = run(x, centroids)
    return ids, xc
